# revision 12
# baseline (speedup 1.0000x reference)
"""Causal single-head attention (B=8, S=E=1024, fp32) for 8 Trainium2 cores.

Strategy: data-parallel over batch — core b handles batch element b.
Host pre-transposes x[b] -> xT [E,S] and the three projection weights
W -> W.T [E,O] so every matmul operand already has the contraction dim
on SBUF partitions. All big matmuls run as float32r (full PE rate with
fp32 storage). Causal structure skips all k>q score/AV blocks.

Per-core pipeline:
  kT[d,s] = WkT.T @ xT (+bk)      (d on partitions)
  qT[d,s] = WqT.T @ xT * 1/32 (+bq/32)
  V[s,d]  = xT.T @ WvT (+bv)      (s on partitions)
  per q-tile i (128 rows):
    scores = qT_i.T @ kT   [128, (i+1)*128] in PSUM
    diag block += causal additive mask
    negmax = -rowmax(scores);  p = exp(scores + negmax) with fused rowsum l
    pT_j = TensorE-transpose of p blocks j<=i
    out_i = (sum_j pT_j.T @ V_j) * (1/l)

Startup: x[eo] and wk-chunk0[eo] DMAs interleave on the sync HWDGE ring
so matmuls begin ~2us in; the first kT weight-chunk runs eo-outer across
8 simultaneously-open PSUM banks so the PE paces with DMA arrival.
"""

import os
import sys
from contextlib import ExitStack

for _p in ("/opt/trn_rl_repo", "/root/.axon_site/_ro/trn_rl_repo"):
    if os.path.isdir(_p) and _p not in sys.path:
        sys.path.insert(0, _p)

import numpy as np

import concourse.bass as bass
import concourse.mybir as mybir
import concourse.tile as tile
from concourse import bacc
from concourse.bass_utils import run_bass_kernel_spmd
from concourse.masks import make_causal_mask, make_identity

P = 128
S = 1024
E = 1024
D = 1024
B = 8
SO = S // P
EO = E // P
DO = D // P
CH = 512
NCH = D // CH
SCALE = 1.0 / np.sqrt(float(E))  # 1/32
MASK_VAL = -1e9

F32 = mybir.dt.float32
F32R = mybir.dt.float32r


def build_program():
    nc = bacc.Bacc(
        "TRN2", target_bir_lowering=False, debug=False, enable_asserts=True
    )

    xT = nc.dram_tensor("xT", [E, S], F32R, kind="ExternalInput").ap()
    wqT = nc.dram_tensor("wqT", [E, D], F32R, kind="ExternalInput").ap()
    wkT = nc.dram_tensor("wkT", [E, D], F32R, kind="ExternalInput").ap()
    wvT = nc.dram_tensor("wvT", [E, D], F32R, kind="ExternalInput").ap()
    bqs = nc.dram_tensor("bqs", [D], F32, kind="ExternalInput").ap()  # bq/32
    bk = nc.dram_tensor("bk", [D], F32, kind="ExternalInput").ap()
    bv = nc.dram_tensor("bv", [D], F32, kind="ExternalInput").ap()
    out = nc.dram_tensor("out", [S, D], F32, kind="ExternalOutput").ap()

    with tile.TileContext(nc) as tc, ExitStack() as ctx:
        consts = ctx.enter_context(tc.tile_pool(name="consts", bufs=1))
        bigs = ctx.enter_context(tc.tile_pool(name="bigs", bufs=1))
        wpool = ctx.enter_context(tc.tile_pool(name="wpool", bufs=2))
        small = ctx.enter_context(tc.tile_pool(name="small", bufs=4))

        # resident tensors
        x_sb = bigs.tile([P, EO, S], F32R)
        kT_sb = bigs.tile([P, DO, S], F32R)
        qT_sb = bigs.tile([P, DO, S], F32R)
        v_sb = bigs.tile([P, SO, D], F32R)

        wq_r = wqT.rearrange("(eo p) o -> p eo o", p=P)
        wk_r = wkT.rearrange("(eo p) o -> p eo o", p=P)
        wv_r = wvT.rearrange("(eo p) o -> p eo o", p=P)

        # ---- startup: interleave x[eo] / wk-chunk0[eo] on the sync ring ----
        wk0_pool = ctx.enter_context(tc.tile_pool(name="wk0_pool", bufs=1))
        wk0 = wk0_pool.tile([P, EO, CH], F32R, name="wk0")
        for eo2 in range(0, EO, 2):
            nc.sync.dma_start(
                x_sb[:, eo2 : eo2 + 2, :],
                xT[eo2 * P : (eo2 + 2) * P, :].rearrange("(t p) s -> p t s", p=P),
            )
            nc.sync.dma_start(wk0[:, eo2 : eo2 + 2, :], wk_r[:, eo2 : eo2 + 2, 0:CH])

        # small consts on the scalar ring (needed by ~first eviction)
        bq_t = consts.tile([P, DO], F32)
        nc.scalar.dma_start(bq_t, bqs.rearrange("(o p) -> p o", p=P))
        bk_t = consts.tile([P, DO], F32)
        nc.scalar.dma_start(bk_t, bk.rearrange("(o p) -> p o", p=P))
        identity = consts.tile([P, P], F32)
        make_identity(nc, identity)
        cmask = consts.tile([P, P], F32)
        make_causal_mask(nc, cmask, mask_val=MASK_VAL)
        # bv broadcast across partitions (needed only for V evictions, late)
        bv_b = consts.tile([P, D], F32)
        nc.scalar.dma_start(bv_b, bv[None, :].broadcast_to([P, D]))

        # ---- kT chunk 0: eo-outer over 8 simultaneously-open psum banks ----
        with tc.tile_pool(name="boot_ps", bufs=8, space="PSUM") as boot_ps:
            groups = [(dj, ch) for dj in range(CH // P) for ch in range(S // CH)]
            boot_tiles = [
                boot_ps.tile([P, CH], F32, tag="boot", name=f"bps_{g}")
                for g in range(len(groups))
            ]
            for eo in range(EO):
                for g, (dj, ch) in enumerate(groups):
                    nc.tensor.matmul(
                        boot_tiles[g],
                        lhsT=wk0[:, eo, dj * P : (dj + 1) * P],
                        rhs=x_sb[:, eo, ch * CH : (ch + 1) * CH],
                        start=(eo == 0),
                        stop=(eo == EO - 1),
                    )
            for g, (dj, ch) in enumerate(groups):
                if g % 2 == 0:
                    nc.scalar.activation(
                        kT_sb[:, dj, ch * CH : (ch + 1) * CH],
                        boot_tiles[g],
                        mybir.ActivationFunctionType.Identity,
                        bias=bk_t[:, dj : dj + 1],
                        scale=1.0,
                    )
                else:
                    nc.vector.tensor_scalar(
                        kT_sb[:, dj, ch * CH : (ch + 1) * CH],
                        boot_tiles[g],
                        bk_t[:, dj : dj + 1],
                        None,
                        mybir.AluOpType.add,
                    )

        ppool = ctx.enter_context(tc.tile_pool(name="ppool", bufs=2))
        acc_ps = ctx.enter_context(tc.tile_pool(name="acc_ps", bufs=3, space="PSUM"))
        sc_ps = ctx.enter_context(tc.tile_pool(name="sc_ps", bufs=2, space="PSUM"))
        tr_ps = ctx.enter_context(tc.tile_pool(name="tr_ps", bufs=1, space="PSUM"))

        def load_w_chunk(w_r, c, nm):
            wt = wpool.tile([P, EO, CH], F32R, tag="wchunk", name=nm)
            nc.sync.dma_start(wt, w_r[:, :, c * CH : (c + 1) * CH])
            return wt

        def project_chunk(wt, c, dst, bias_t, scale):
            # dst[d_part, do, s] (+bias per-partition), for d in chunk c
            for dj in range(CH // P):
                do = c * (CH // P) + dj
                for ch in range(S // CH):
                    ps = acc_ps.tile([P, CH], F32, tag="acc", name="ps")
                    for eo in range(EO):
                        nc.tensor.matmul(
                            ps,
                            lhsT=wt[:, eo, dj * P : (dj + 1) * P],
                            rhs=x_sb[:, eo, ch * CH : (ch + 1) * CH],
                            start=(eo == 0),
                            stop=(eo == EO - 1),
                        )
                    nc.scalar.activation(
                        dst[:, do, ch * CH : (ch + 1) * CH],
                        ps,
                        mybir.ActivationFunctionType.Identity,
                        bias=bias_t[:, do : do + 1],
                        scale=scale,
                    )

        # rest of kT, then qT (attention needs them first)
        wt = load_w_chunk(wk_r, 1, "wk1")
        project_chunk(wt, 1, kT_sb, bk_t, 1.0)
        for c in range(NCH):
            wt = load_w_chunk(wq_r, c, f"wq{c}")
            project_chunk(wt, c, qT_sb, bq_t, SCALE)

        # V[s_part, so, d] = x.T @ WvT (+bv along free dim)
        for c in range(NCH):
            wt = load_w_chunk(wv_r, c, f"wv{c}")
            for so in range(SO):
                ps = acc_ps.tile([P, CH], F32, tag="acc", name="ps")
                for eo in range(EO):
                    nc.tensor.matmul(
                        ps,
                        lhsT=x_sb[:, eo, so * P : (so + 1) * P],
                        rhs=wt[:, eo, :],
                        start=(eo == 0),
                        stop=(eo == EO - 1),
                    )
                nc.vector.tensor_tensor(
                    v_sb[:, so, c * CH : (c + 1) * CH],
                    ps,
                    bv_b[:, c * CH : (c + 1) * CH],
                    mybir.AluOpType.add,
                )

        # ---- attention per q-tile ----
        for i in range(SO):
            nk = i + 1
            kw = nk * P
            ps_s = sc_ps.tile([P, S], F32, tag="scores", name="ps_s")
            nfull = kw // CH
            rem = kw - nfull * CH
            for ch in range(nfull + (1 if rem else 0)):
                w = CH if ch < nfull else rem
                for do in range(DO):
                    nc.tensor.matmul(
                        ps_s[:, ch * CH : ch * CH + w],
                        lhsT=qT_sb[:, do, i * P : (i + 1) * P],
                        rhs=kT_sb[:, do, ch * CH : ch * CH + w],
                        start=(do == 0),
                        stop=(do == DO - 1),
                    )
            # additive causal mask on the diagonal block
            nc.vector.tensor_tensor(
                ps_s[:, i * P : (i + 1) * P],
                ps_s[:, i * P : (i + 1) * P],
                cmask,
                mybir.AluOpType.add,
            )
            negmax = small.tile([P, 1], F32, tag="negmax", name="negmax")
            nc.vector.tensor_reduce(
                negmax,
                ps_s[:, :kw],
                axis=mybir.AxisListType.X,
                op=mybir.AluOpType.max,
                negate=True,
            )
            p_sb = ppool.tile([P, S], F32, tag="p", name="p_sb")
            lsum = small.tile([P, 1], F32, tag="lsum", name="lsum")
            nc.scalar.activation(
                p_sb[:, :kw],
                ps_s[:, :kw],
                mybir.ActivationFunctionType.Exp,
                bias=negmax,
                scale=1.0,
                accum_out=lsum,
            )
            rinv = small.tile([P, 1], F32, tag="rinv", name="rinv")
            nc.vector.reciprocal(rinv, lsum)

            pT = ppool.tile([P, S], F32R, tag="pT", name="pT")
            for j in range(nk):
                ps_t = tr_ps.tile([P, P], F32, tag="tr", name="ps_t")
                nc.tensor.transpose(ps_t, p_sb[:, j * P : (j + 1) * P], identity)
                nc.vector.tensor_copy(pT[:, j * P : (j + 1) * P], ps_t)

            out_sb = ppool.tile([P, D], F32, tag="out", name="out_sb")
            for c2 in range(NCH):
                ps_o = acc_ps.tile([P, CH], F32, tag="acc", name="ps_o")
                for j in range(nk):
                    nc.tensor.matmul(
                        ps_o,
                        lhsT=pT[:, j * P : (j + 1) * P],
                        rhs=v_sb[:, j, c2 * CH : (c2 + 1) * CH],
                        start=(j == 0),
                        stop=(j == nk - 1),
                    )
                nc.vector.tensor_scalar_mul(
                    out_sb[:, c2 * CH : (c2 + 1) * CH], ps_o, rinv
                )
                nc.scalar.dma_start(
                    out[i * P : (i + 1) * P, c2 * CH : (c2 + 1) * CH],
                    out_sb[:, c2 * CH : (c2 + 1) * CH],
                )

    nc.compile()
    return nc


_NC_CACHE = None


def get_program():
    global _NC_CACHE
    if _NC_CACHE is None:
        _NC_CACHE = build_program()
    return _NC_CACHE


def make_in_maps(x, Wq, bq, Wk, bk, Wv, bv):
    x = np.ascontiguousarray(np.asarray(x, dtype=np.float32))
    wqT = np.ascontiguousarray(np.asarray(Wq, dtype=np.float32).T)
    wkT = np.ascontiguousarray(np.asarray(Wk, dtype=np.float32).T)
    wvT = np.ascontiguousarray(np.asarray(Wv, dtype=np.float32).T)
    bqs = np.asarray(bq, dtype=np.float32) * np.float32(SCALE)
    bk = np.asarray(bk, dtype=np.float32)
    bv = np.asarray(bv, dtype=np.float32)
    in_maps = []
    for b in range(B):
        in_maps.append(
            {
                "xT": np.ascontiguousarray(x[b].T),
                "wqT": wqT,
                "wkT": wkT,
                "wvT": wvT,
                "bqs": bqs,
                "bk": bk,
                "bv": bv,
            }
        )
    return in_maps


def run_on_hw(in_maps, trace=False, **kwargs):
    nc = get_program()
    return run_bass_kernel_spmd(
        nc, in_maps, core_ids=list(range(B)), trace=trace, **kwargs
    )


def kernel(x, Wq, bq, Wk, bk, Wv, bv):
    in_maps = make_in_maps(x, Wq, bq, Wk, bk, Wv, bv)
    res = run_on_hw(in_maps)
    return np.stack([res.results[b]["out"] for b in range(B)], axis=0)


# revision 13
# speedup vs baseline: 1.0483x; 1.0483x over previous
"""Causal single-head attention (B=8, S=E=1024, fp32) for 8 Trainium2 cores.

Strategy: data-parallel over batch — core b handles batch element b.
Host pre-transposes x[b] -> xT [E,S] and the three projection weights
W -> W.T [E,O] so every matmul operand already has the contraction dim
on SBUF partitions. All big matmuls run as float32r (full PE rate with
fp32 storage). Causal structure skips all k>q score/AV blocks.

Per-core pipeline:
  kT[d,s] = WkT.T @ xT (+bk)      (d on partitions)
  qT[d,s] = WqT.T @ xT * 1/32 (+bq/32)
  V[s,d]  = xT.T @ WvT (+bv)      (s on partitions)
  per q-tile i (128 rows):
    scores = qT_i.T @ kT   [128, (i+1)*128] in PSUM
    diag block += causal additive mask
    negmax = -rowmax(scores);  p = exp(scores + negmax) with fused rowsum l
    pT_j = TensorE-transpose of p blocks j<=i
    out_i = (sum_j pT_j.T @ V_j) * (1/l)

Startup: x[eo] and wk-chunk0[eo] DMAs interleave on the sync HWDGE ring
so matmuls begin ~2us in; the first kT weight-chunk runs eo-outer across
8 simultaneously-open PSUM banks so the PE paces with DMA arrival.
"""

import os
import sys
from contextlib import ExitStack

for _p in ("/opt/trn_rl_repo", "/root/.axon_site/_ro/trn_rl_repo"):
    if os.path.isdir(_p) and _p not in sys.path:
        sys.path.insert(0, _p)

import numpy as np

import concourse.bass as bass
import concourse.mybir as mybir
import concourse.tile as tile
from concourse import bacc
from concourse.bass_utils import run_bass_kernel_spmd
from concourse.masks import make_causal_mask, make_identity

P = 128
S = 1024
E = 1024
D = 1024
B = 8
SO = S // P
EO = E // P
DO = D // P
CH = 512
NCH = D // CH
SCALE = 1.0 / np.sqrt(float(E))  # 1/32
MASK_VAL = -1e9

F32 = mybir.dt.float32
F32R = mybir.dt.float32r


def build_program():
    nc = bacc.Bacc(
        "TRN2", target_bir_lowering=False, debug=False, enable_asserts=True
    )

    xT = nc.dram_tensor("xT", [E, S], F32R, kind="ExternalInput").ap()
    wqT = nc.dram_tensor("wqT", [E, D], F32R, kind="ExternalInput").ap()
    wkT = nc.dram_tensor("wkT", [E, D], F32R, kind="ExternalInput").ap()
    wvT = nc.dram_tensor("wvT", [E, D], F32R, kind="ExternalInput").ap()
    bqs = nc.dram_tensor("bqs", [D], F32, kind="ExternalInput").ap()  # bq/32
    bk = nc.dram_tensor("bk", [D], F32, kind="ExternalInput").ap()
    bv = nc.dram_tensor("bv", [D], F32, kind="ExternalInput").ap()
    out = nc.dram_tensor("out", [S, D], F32, kind="ExternalOutput").ap()

    with tile.TileContext(nc) as tc, ExitStack() as ctx:
        consts = ctx.enter_context(tc.tile_pool(name="consts", bufs=1))
        bigs = ctx.enter_context(tc.tile_pool(name="bigs", bufs=1))
        wpool = ctx.enter_context(tc.tile_pool(name="wpool", bufs=2))
        small = ctx.enter_context(tc.tile_pool(name="small", bufs=4))

        # resident tensors
        x_sb = bigs.tile([P, EO, S], F32R)
        kT_sb = bigs.tile([P, DO, S], F32R)
        qT_sb = bigs.tile([P, DO, S], F32R)
        v_sb = bigs.tile([P, SO, D], F32R)

        wq_r = wqT.rearrange("(eo p) o -> p eo o", p=P)
        wk_r = wkT.rearrange("(eo p) o -> p eo o", p=P)
        wv_r = wvT.rearrange("(eo p) o -> p eo o", p=P)

        # ---- startup: interleave x[eo] / wk-chunk0[eo] on the sync ring ----
        wk0_pool = ctx.enter_context(tc.tile_pool(name="wk0_pool", bufs=1))
        wk0 = wk0_pool.tile([P, EO, CH], F32R, name="wk0")
        for eo2 in range(0, EO, 2):
            nc.sync.dma_start(
                x_sb[:, eo2 : eo2 + 2, :],
                xT[eo2 * P : (eo2 + 2) * P, :].rearrange("(t p) s -> p t s", p=P),
            )
            nc.sync.dma_start(wk0[:, eo2 : eo2 + 2, :], wk_r[:, eo2 : eo2 + 2, 0:CH])

        # small consts on the scalar ring (needed by ~first eviction)
        bq_t = consts.tile([P, DO], F32)
        nc.scalar.dma_start(bq_t, bqs.rearrange("(o p) -> p o", p=P))
        bk_t = consts.tile([P, DO], F32)
        nc.scalar.dma_start(bk_t, bk.rearrange("(o p) -> p o", p=P))
        identity = consts.tile([P, P], F32)
        make_identity(nc, identity)
        cmask = consts.tile([P, P], F32)
        make_causal_mask(nc, cmask, mask_val=MASK_VAL)
        # bv broadcast across partitions (needed only for V evictions, late)
        bv_b = consts.tile([P, D], F32)
        nc.scalar.dma_start(bv_b, bv[None, :].broadcast_to([P, D]))

        # ---- kT chunk 0: eo-outer over 8 simultaneously-open psum banks ----
        with tc.tile_pool(name="boot_ps", bufs=8, space="PSUM") as boot_ps:
            groups = [(dj, ch) for dj in range(CH // P) for ch in range(S // CH)]
            boot_tiles = [
                boot_ps.tile([P, CH], F32, tag="boot", name=f"bps_{g}")
                for g in range(len(groups))
            ]
            for eo in range(EO):
                for g, (dj, ch) in enumerate(groups):
                    nc.tensor.matmul(
                        boot_tiles[g],
                        lhsT=wk0[:, eo, dj * P : (dj + 1) * P],
                        rhs=x_sb[:, eo, ch * CH : (ch + 1) * CH],
                        start=(eo == 0),
                        stop=(eo == EO - 1),
                    )
            for g, (dj, ch) in enumerate(groups):
                if g % 2 == 0:
                    nc.scalar.activation(
                        kT_sb[:, dj, ch * CH : (ch + 1) * CH],
                        boot_tiles[g],
                        mybir.ActivationFunctionType.Identity,
                        bias=bk_t[:, dj : dj + 1],
                        scale=1.0,
                    )
                else:
                    nc.vector.tensor_scalar(
                        kT_sb[:, dj, ch * CH : (ch + 1) * CH],
                        boot_tiles[g],
                        bk_t[:, dj : dj + 1],
                        None,
                        mybir.AluOpType.add,
                    )

        ppool = ctx.enter_context(tc.tile_pool(name="ppool", bufs=2))
        acc_ps = ctx.enter_context(tc.tile_pool(name="acc_ps", bufs=2, space="PSUM"))
        sc_ps = ctx.enter_context(tc.tile_pool(name="sc_ps", bufs=2, space="PSUM"))
        tr_ps = ctx.enter_context(tc.tile_pool(name="tr_ps", bufs=2, space="PSUM"))

        def load_w_chunk(w_r, c, nm):
            wt = wpool.tile([P, EO, CH], F32R, tag="wchunk", name=nm)
            nc.sync.dma_start(wt, w_r[:, :, c * CH : (c + 1) * CH])
            return wt

        def project_chunk(wt, c, dst, bias_t, scale):
            # dst[d_part, do, s] (+bias per-partition), for d in chunk c
            for dj in range(CH // P):
                do = c * (CH // P) + dj
                for ch in range(S // CH):
                    ps = acc_ps.tile([P, CH], F32, tag="acc", name="ps")
                    for eo in range(EO):
                        nc.tensor.matmul(
                            ps,
                            lhsT=wt[:, eo, dj * P : (dj + 1) * P],
                            rhs=x_sb[:, eo, ch * CH : (ch + 1) * CH],
                            start=(eo == 0),
                            stop=(eo == EO - 1),
                        )
                    nc.scalar.activation(
                        dst[:, do, ch * CH : (ch + 1) * CH],
                        ps,
                        mybir.ActivationFunctionType.Identity,
                        bias=bias_t[:, do : do + 1],
                        scale=scale,
                    )

        # rest of kT, then qT (attention needs them first)
        wt = load_w_chunk(wk_r, 1, "wk1")
        project_chunk(wt, 1, kT_sb, bk_t, 1.0)
        for c in range(NCH):
            wt = load_w_chunk(wq_r, c, f"wq{c}")
            project_chunk(wt, c, qT_sb, bq_t, SCALE)

        # V[s_part, so, d] = x.T @ WvT (+bv along free dim)
        for c in range(NCH):
            wt = load_w_chunk(wv_r, c, f"wv{c}")
            for so in range(SO):
                ps = acc_ps.tile([P, CH], F32, tag="acc", name="ps")
                for eo in range(EO):
                    nc.tensor.matmul(
                        ps,
                        lhsT=x_sb[:, eo, so * P : (so + 1) * P],
                        rhs=wt[:, eo, :],
                        start=(eo == 0),
                        stop=(eo == EO - 1),
                    )
                nc.vector.tensor_tensor(
                    v_sb[:, so, c * CH : (c + 1) * CH],
                    ps,
                    bv_b[:, c * CH : (c + 1) * CH],
                    mybir.AluOpType.add,
                )

        # ---- attention per q-tile ----
        for i in range(SO):
            nk = i + 1
            kw = nk * P
            ps_s = sc_ps.tile([P, S], F32, tag="scores", name="ps_s")
            nfull = kw // CH
            rem = kw - nfull * CH
            for ch in range(nfull + (1 if rem else 0)):
                w = CH if ch < nfull else rem
                for do in range(DO):
                    nc.tensor.matmul(
                        ps_s[:, ch * CH : ch * CH + w],
                        lhsT=qT_sb[:, do, i * P : (i + 1) * P],
                        rhs=kT_sb[:, do, ch * CH : ch * CH + w],
                        start=(do == 0),
                        stop=(do == DO - 1),
                    )
            # additive causal mask on the diagonal block
            nc.vector.tensor_tensor(
                ps_s[:, i * P : (i + 1) * P],
                ps_s[:, i * P : (i + 1) * P],
                cmask,
                mybir.AluOpType.add,
            )
            negmax = small.tile([P, 1], F32, tag="negmax", name="negmax")
            nc.vector.tensor_reduce(
                negmax,
                ps_s[:, :kw],
                axis=mybir.AxisListType.X,
                op=mybir.AluOpType.max,
                negate=True,
            )
            p_sb = ppool.tile([P, S], F32, tag="p", name="p_sb")
            lsum = small.tile([P, 1], F32, tag="lsum", name="lsum")
            nc.scalar.activation(
                p_sb[:, :kw],
                ps_s[:, :kw],
                mybir.ActivationFunctionType.Exp,
                bias=negmax,
                scale=1.0,
                accum_out=lsum,
            )
            rinv = small.tile([P, 1], F32, tag="rinv", name="rinv")
            nc.vector.reciprocal(rinv, lsum)

            pT = ppool.tile([P, S], F32R, tag="pT", name="pT")
            for j in range(nk):
                ps_t = tr_ps.tile([P, P], F32, tag="tr", name="ps_t")
                nc.tensor.transpose(ps_t, p_sb[:, j * P : (j + 1) * P], identity)
                nc.vector.tensor_copy(pT[:, j * P : (j + 1) * P], ps_t)

            out_sb = ppool.tile([P, D], F32, tag="out", name="out_sb")
            for c2 in range(NCH):
                ps_o = acc_ps.tile([P, CH], F32, tag="acc", name="ps_o")
                for j in range(nk):
                    nc.tensor.matmul(
                        ps_o,
                        lhsT=pT[:, j * P : (j + 1) * P],
                        rhs=v_sb[:, j, c2 * CH : (c2 + 1) * CH],
                        start=(j == 0),
                        stop=(j == nk - 1),
                    )
                nc.vector.tensor_scalar_mul(
                    out_sb[:, c2 * CH : (c2 + 1) * CH], ps_o, rinv
                )
                nc.scalar.dma_start(
                    out[i * P : (i + 1) * P, c2 * CH : (c2 + 1) * CH],
                    out_sb[:, c2 * CH : (c2 + 1) * CH],
                )

    nc.compile()
    return nc


_NC_CACHE = None


def get_program():
    global _NC_CACHE
    if _NC_CACHE is None:
        _NC_CACHE = build_program()
    return _NC_CACHE


def make_in_maps(x, Wq, bq, Wk, bk, Wv, bv):
    x = np.ascontiguousarray(np.asarray(x, dtype=np.float32))
    wqT = np.ascontiguousarray(np.asarray(Wq, dtype=np.float32).T)
    wkT = np.ascontiguousarray(np.asarray(Wk, dtype=np.float32).T)
    wvT = np.ascontiguousarray(np.asarray(Wv, dtype=np.float32).T)
    bqs = np.asarray(bq, dtype=np.float32) * np.float32(SCALE)
    bk = np.asarray(bk, dtype=np.float32)
    bv = np.asarray(bv, dtype=np.float32)
    in_maps = []
    for b in range(B):
        in_maps.append(
            {
                "xT": np.ascontiguousarray(x[b].T),
                "wqT": wqT,
                "wkT": wkT,
                "wvT": wvT,
                "bqs": bqs,
                "bk": bk,
                "bv": bv,
            }
        )
    return in_maps


def run_on_hw(in_maps, trace=False, **kwargs):
    nc = get_program()
    return run_bass_kernel_spmd(
        nc, in_maps, core_ids=list(range(B)), trace=trace, **kwargs
    )


def kernel(x, Wq, bq, Wk, bk, Wv, bv):
    in_maps = make_in_maps(x, Wq, bq, Wk, bk, Wv, bv)
    res = run_on_hw(in_maps)
    return np.stack([res.results[b]["out"] for b in range(B)], axis=0)
